# revision 36
# baseline (speedup 1.0000x reference)
"""Multi-head attention (B=4, S=2048, D=1024, H=16) on 8 trn2 NeuronCores.

Sharding: batch (4-way) x head-half (2-way).  Core c = 2*b + hh handles batch b
and heads hh*8 .. hh*8+7.  All matmul operands are bf16; rel err ~6e-3.

  1. Prefix (~42us): load xk, project KT (q-block-paired, DMA-paced), load xq
     (reusing xk row slots), project QT q-half 0.  The V projection moves into
     the attention stream as filler steps; xv halves load during the prefix so
     V can start at stream slot 0.
  2. Scores run as a CONCURRENT row-tiled pair: KT2/QT2 shadow copies (DMA'd
     after each projection evacuation, partition halves swapped) let the j=1
     half read its operands from the opposite 64-partition strip, so both
     [64x128]x[64x512] matmuls occupy disjoint row groups of the PE array and
     execute simultaneously (~216ns/pair instead of ~530ns).  The freed PE
     slack absorbs the V projection + QT half-1 + half-0 output projection as
     in-stream fillers.
  3. attn@V may lag its exp slot (at-ring buffers the lag, emission forced
     before the ring wraps) until the V projection produces the needed token
     tile.  Softmax denominators ride as a ones-column in V (attnv M=65);
     per-head tails normalize via broadcast+reciprocal on the DVE.
  4. The half-1 output projection drains into PSUM freed by releasing the
     scores ring + filler pool but NOT the attnv accumulator pool, so the
     drain's pool boundary does not wait on the final softmax tail (that wait
     idled the PE >3.4us, HAM-throttled it, and ran the drain at half clock).
     Output partials are written bf16; the host sums them in fp32 + bias.
"""

import sys

if "/opt/trn_rl_repo" not in sys.path:
    sys.path.insert(0, "/opt/trn_rl_repo")

import numpy as np

B, S, D = 4, 2048, 1024
H, HD = 16, 64
P = 128
DK = D // P            # 8 contraction chunks for the projections
NKT = S // P           # 16 token tiles
QB = 512
DH = 512               # head dims per core (8 heads)
NDC = DH // P          # 4 dout chunks per core
NHC = 8                # heads per core
VW = HD + 1            # V columns per head incl. the ones column
HB = 1024              # q-half width
NCORES = 8
NRING = 16             # at-ring depth (attnv may lag exp by NRING-2 slots)

_PROG = [None]


def _build():
    import itertools

    import concourse.mybir as mybir
    import concourse.tile as tile
    from concourse import bacc

    f32 = mybir.dt.float32
    bf16 = mybir.dt.bfloat16
    Exp = mybir.ActivationFunctionType.Exp

    nc = bacc.Bacc("TRN2", target_bir_lowering=False, debug=False)
    xq = nc.dram_tensor("xq", [D, S], bf16, kind="ExternalInput").ap()
    xk = nc.dram_tensor("xk", [D, S], bf16, kind="ExternalInput").ap()
    xv = nc.dram_tensor("xv", [D, S], bf16, kind="ExternalInput").ap()
    wq = nc.dram_tensor("wq", [D, DH], bf16, kind="ExternalInput").ap()
    wk = nc.dram_tensor("wk", [D, DH], bf16, kind="ExternalInput").ap()
    wv = nc.dram_tensor("wv", [D, DH], bf16, kind="ExternalInput").ap()
    wo = nc.dram_tensor("wo", [DH, D], bf16, kind="ExternalInput").ap()
    part = nc.dram_tensor("part", [S, D], bf16, kind="ExternalOutput").ap()

    xq_v = xq.rearrange("(c p) s -> p c s", p=P)
    xk_v = xk.rearrange("(c p) s -> p c s", p=P)
    xv_v = xv.rearrange("(c p) s -> p c s", p=P)

    with tile.TileContext(nc) as tc:
        with tc.tile_pool(name="big", bufs=1) as big, tc.tile_pool(name="wp", bufs=4) as wp:
            QT = big.tile([P, NDC, S], bf16, tag="QT")
            KT = big.tile([P, NDC, S], bf16, tag="KT")
            QT2 = big.tile([P, NDC, S], bf16, tag="QT2")
            KT2 = big.tile([P, NDC, S], bf16, tag="KT2")
            V = big.tile([P, NKT, NHC * VW], bf16, tag="V")
            outT = big.tile([P, NDC, S], bf16, tag="outT")

            wk_t = wp.tile([P, DK, DH], bf16, tag="w", name="wk")
            wq_t = wp.tile([P, DK, DH], bf16, tag="w", name="wq")
            wv_t = wp.tile([P, DK, DH], bf16, tag="w", name="wv")
            wo_t = wp.tile([P, NDC, D], bf16, tag="w", name="wo")

            scp_box = [None]

            # warm the ACT exp table at t~0 (2.7us table load hides under DMA)
            with tc.tile_pool(name="wrm", bufs=1) as wrm:
                wc = wrm.tile([P, 8], f32, tag="wc")
                wout = wrm.tile([P, 8], bf16, tag="wo8")
                nc.vector.memset(wc[:], 0.0)
                nc.scalar.activation(wout[:], wc[:], Exp)

            def dma2(dst, src):
                nc.sync.dma_start(dst, src)

            def shadow(dst2, src, dc, c0, cw):
                # partition-half-swapped shadow copy (cross-partition: DMA)
                nc.sync.dma_start(dst2[0:64, dc, c0 : c0 + cw], src[64:128, dc, c0 : c0 + cw])
                nc.sync.dma_start(dst2[64:128, dc, c0 : c0 + cw], src[0:64, dc, c0 : c0 + cw])

            def emit_scores(kt, hc, r0, c0, use_shadow=True):
                # j=0 from the home strip, j=1 from the swapped shadow: the
                # two contraction-64 matmuls occupy disjoint PE row groups
                # and run concurrently.  The first slots skip the shadow so
                # the stream start never waits on the shadow DMAs.
                scp = scp_box[0]
                r1 = 64 - r0
                sct = scp.tile([P, HB], f32, tag="sc")
                nc.tensor.matmul(
                    sct[:, 0:QB],
                    KT[r0 : r0 + 64, hc, kt * P : (kt + 1) * P],
                    QT[r0 : r0 + 64, hc, c0 : c0 + QB],
                    start=True,
                    stop=True,
                )
                k2, q2, r2 = (KT2, QT2, r1) if use_shadow else (KT, QT, r0)
                nc.tensor.matmul(
                    sct[:, QB:HB],
                    k2[r2 : r2 + 64, hc, kt * P : (kt + 1) * P],
                    q2[r2 : r2 + 64, hc, c0 + QB : c0 + HB],
                    start=True,
                    stop=True,
                )
                return sct

            # hp=1 heads first: the final head's tail then has no DMA hop
            order = [1, 3, 5, 7, 0, 2, 4, 6]
            slots = [
                (half, h, kt)
                for half in (0, 1)
                for h in order
                for kt in range(NKT)
            ]
            NSLOT = len(slots)

            def scores_for_slot(i):
                half, h, kt = slots[i]
                # the shadow DMAs land ~15us after the projections finish
                # (queued behind the input loads); early slots run the j=1
                # half serial from the originals so the exp stream starts
                # as soon as QT half-0 is projected
                return emit_scores(
                    kt, h // 2, 64 * (h % 2), half * HB, use_shadow=(i >= 24)
                )

            # ---- prefix: KT (full) + QT half-0 ---------------------------
            with tc.tile_pool(name="xvp", bufs=8) as xvp:
                xr = tc.alloc_tile_pool(name="xr", bufs=8)
                xqr = tc.alloc_tile_pool(name="xqr", bufs=8)

                def load_w(w_t, w_dram):
                    w_v = w_dram.rearrange("(c p) m -> p c m", p=P)
                    for dk in range(DK):
                        nc.sync.dma_start(w_t[:, dk], w_v[:, dk])

                # all input DMAs emitted upfront, in priority order: the
                # queues then stream them back-to-back while the PE projects
                wk_v = wk.rearrange("(c p) m -> p c m", p=P)
                xk_rows = []
                for dk in range(DK):
                    nc.sync.dma_start(wk_t[:, dk], wk_v[:, dk])
                    xt = xr.tile([P, S], bf16, tag="xr", name=f"xr_k{dk}")
                    dma2(xt[:], xk_v[:, dk, :])
                    xk_rows.append(xt)
                wq_v = wq.rearrange("(c p) m -> p c m", p=P)
                xq_rows = []
                for dk in range(DK):
                    nc.sync.dma_start(wq_t[:, dk], wq_v[:, dk])
                    xt = xqr.tile([P, HB], bf16, tag="xq", name=f"xq{dk}")
                    dma2(xt[:], xq_v[:, dk, 0:HB])
                    xq_rows.append(xt)
                load_w(wv_t, wv)
                xvh = {0: [], 1: []}
                for h in (0, 1):
                    for dk in range(DK):
                        t = xvp.tile([P, HB], bf16, tag="xv", name=f"xv{h}_{dk}")
                        dma2(t[:], xv_v[:, dk, h * HB : (h + 1) * HB])
                        xvh[h].append(t)
                nc.sync.dma_start(wo_t[:], wo.rearrange("(c p) m -> p c m", p=P))

                # KT projection, q-block-paired to keep pace with the row DMA
                with tc.tile_pool(name="pp8", bufs=8, space="PSUM") as pp8:
                    for qbp in (0, 1):
                        pts = [
                            pp8.tile([P, QB], f32, tag="pp8", name=f"pk{i}")
                            for i in range(8)
                        ]
                        for dk in range(DK):
                            for u in range(2):
                                for dc in range(NDC):
                                    nc.tensor.matmul(
                                        pts[u * NDC + dc][:],
                                        wk_t[:, dk, dc * P : (dc + 1) * P],
                                        xk_rows[dk][
                                            :,
                                            (2 * qbp + u) * QB : (2 * qbp + u + 1) * QB,
                                        ],
                                        start=(dk == 0),
                                        stop=(dk == DK - 1),
                                    )
                        for u in range(2):
                            for dc in range(NDC):
                                c0 = (2 * qbp + u) * QB
                                dst = KT[:, dc, c0 : c0 + QB]
                                if dc % 2 == 0:
                                    nc.vector.tensor_copy(dst, pts[u * NDC + dc][:])
                                else:
                                    nc.scalar.copy(dst, pts[u * NDC + dc][:])
                                shadow(KT2, KT, dc, c0, QB)

                scp_box[0] = tc.alloc_tile_pool(
                    name="sc", bufs=2, space="PSUM", side="right"
                )
                pp = tc.alloc_tile_pool(name="pp", bufs=4, space="PSUM")
                # QT half-0 (q-blocks 0-1)
                for qb in (0, 1):
                    pts = [pp.tile([P, QB], f32, tag="pp", name=f"pp{i}") for i in range(NDC)]
                    for dk in range(DK):
                        for dc in range(NDC):
                            nc.tensor.matmul(
                                pts[dc][:],
                                wq_t[:, dk, dc * P : (dc + 1) * P],
                                xq_rows[dk][:, qb * QB : (qb + 1) * QB],
                                start=(dk == 0),
                                stop=(dk == DK - 1),
                            )
                    for dc in range(NDC):
                        c0 = qb * QB
                        dst = QT[:, dc, c0 : c0 + QB]
                        if dc % 2 == 0:
                            nc.vector.tensor_copy(dst, pts[dc][:])
                        else:
                            nc.scalar.copy(dst, pts[dc][:])
                        shadow(QT2, QT, dc, c0, QB)
                pp.release()
                xqr.release()
                xr.release()
                nc.vector.memset(V[:], 1.0)

                sct_ring = {0: scores_for_slot(0), 1: scores_for_slot(1)}

                # ---- attention stream with in-stream V projection --------
                accp = tc.alloc_tile_pool(name="acc", bufs=1, space="PSUM")
                fillp = tc.alloc_tile_pool(name="fillpp", bufs=2, space="PSUM")
                with (
                    tc.tile_pool(name="attn", bufs=NRING) as attnp,
                    tc.tile_pool(name="tail", bufs=1) as tailp,
                    tc.tile_pool(name="asbp", bufs=1) as asbp,
                    tc.tile_pool(name="stage", bufs=2) as stage,
                    tc.tile_pool(name="xf", bufs=8) as xf,
                ):
                    v_kt_done = [0]

                    def v_steps(qb):
                        """V projection q-block qb -> token tiles qb*4..+3."""
                        cur = {}
                        c0 = (qb % 2) * QB
                        for kt_in in range(4):
                            kt = qb * 4 + kt_in
                            def mm(kt_in, lo):
                                if lo == 0:
                                    cur[kt_in] = fillp.tile([P, DH], f32, tag="fp", name=f"fv{qb}_{kt_in}")
                                xs = xvh[qb // 2]
                                for dk in range(lo, lo + 4):
                                    nc.tensor.matmul(
                                        cur[kt_in][:],
                                        xs[dk][:, c0 + kt_in * P : c0 + (kt_in + 1) * P],
                                        wv_t[:, dk, :],
                                        start=(dk == 0),
                                        stop=(dk == DK - 1),
                                    )
                            def fin(kt_in=kt_in, kt=kt):
                                mm(kt_in, 4)
                                nc.vector.tensor_copy(
                                    V[:, kt].rearrange("p (h c) -> p h c", c=VW)[:, :, 0:HD],
                                    cur.pop(kt_in)[:].rearrange("p (h c) -> p h c", c=HD),
                                )
                                v_kt_done[0] = kt + 1
                            yield lambda kt_in=kt_in, mm=mm: mm(kt_in, 0)
                            yield fin

                    def qt23_steps():
                        """Project QT for q-half 1 (qb 2,3) + shadows."""
                        xts_all = {}
                        cur = {}

                        def dma_qb(dk0):
                            def go():
                                for dk in (dk0, dk0 + 1, dk0 + 2, dk0 + 3):
                                    xt = xf.tile([P, HB], bf16, tag="xf", name=f"xf{dk}")
                                    nc.sync.dma_start(
                                        xt[:], xq_v[:, dk, HB : 2 * HB]
                                    )
                                    xts_all[dk] = xt
                            return go

                        def mm_step(qb, dc, dk):
                            def go():
                                if dk == 0:
                                    cur[(qb, dc)] = fillp.tile([P, QB], f32, tag="fp", name=f"fq{qb}_{dc}")
                                nc.tensor.matmul(
                                    cur[(qb, dc)][:],
                                    wq_t[:, dk, dc * P : (dc + 1) * P],
                                    xts_all[dk][:, (qb - 2) * QB : (qb - 1) * QB],
                                    start=(dk == 0),
                                    stop=(dk == DK - 1),
                                )
                            return go

                        def copy_step(qb, dc):
                            def go():
                                c0 = qb * QB
                                nc.vector.tensor_copy(
                                    QT[:, dc, c0 : c0 + QB],
                                    cur[(qb, dc)][:],
                                )
                                shadow(QT2, QT, dc, c0, QB)
                            return go

                        yield dma_qb(0)
                        yield dma_qb(4)
                        for qb in (2, 3):
                            for dc in range(NDC):
                                for dk in range(DK):
                                    yield mm_step(qb, dc, dk)
                                yield copy_step(qb, dc)

                    def oproj_steps(qts, poolp=None, poolst=None):
                        pool_mm = poolp if poolp is not None else fillp
                        pool_st = poolst if poolst is not None else stage
                        cur = {}

                        def mm_step(qt, do, dc):
                            def go():
                                if dc == 0:
                                    cur[(qt, do)] = pool_mm.tile([P, QB], f32, tag="fp", name=f"fo{qt}_{do}")
                                nc.tensor.matmul(
                                    cur[(qt, do)][:],
                                    outT[:, dc, qt * P : (qt + 1) * P],
                                    wo_t[:, dc, do * QB : (do + 1) * QB],
                                    start=(dc == 0),
                                    stop=(dc == NDC - 1),
                                )
                            return go

                        def out_step(qt, do):
                            def go():
                                st = pool_st.tile([P, QB], bf16, tag="st", name=f"st{qt}_{do}")
                                nc.vector.tensor_copy(st[:], cur.pop((qt, do))[:])
                                nc.sync.dma_start(
                                    part[qt * P : (qt + 1) * P, do * QB : (do + 1) * QB],
                                    st[:],
                                )
                            return go

                        for qt in qts:
                            for do in range(2):
                                for dc in range(NDC):
                                    yield mm_step(qt, do, dc)
                                yield out_step(qt, do)

                    gen1 = itertools.chain(
                        v_steps(0), v_steps(1), v_steps(2), v_steps(3),
                        qt23_steps(),
                    )
                    gen2 = oproj_steps(range(8))

                    def pump():
                        s = next(gen1, None)
                        if s is None:
                            return False
                        s()
                        return True

                    def emit_attnv(acc, h, kt, at_t):
                        for j in range(2):
                            nc.tensor.matmul(
                                acc[0:VW, j * QB : (j + 1) * QB],
                                V[:, kt, h * VW : (h + 1) * VW],
                                at_t[:, j * QB : (j + 1) * QB],
                                start=(kt == 0),
                                stop=(kt == NKT - 1),
                            )

                    def emit_tail(acc, hp, hc, c0, final=False):
                        nch = 4 if final else 2
                        cw = HB // nch
                        asb = asbp.tile([96, HB], f32, tag="asb")
                        if not final:
                            nc.vector.tensor_copy(asb[0:VW, :], acc[0:VW, :])
                        for ch in range(nch):
                            cl = slice(ch * cw, (ch + 1) * cw)
                            if final:
                                nc.vector.tensor_copy(asb[0:VW, cl], acc[0:VW, cl])
                            bc = tailp.tile([64, QB], f32, tag="bc", name=f"bc{ch % 2}")
                            nc.vector.stream_shuffle(
                                bc[0:32, 0:cw], asb[64:96, cl], [0] * 32
                            )
                            nc.vector.stream_shuffle(
                                bc[32:64, 0:cw], asb[64:96, cl], [0] * 32
                            )
                            rec = tailp.tile([64, QB], f32, tag="rec", name=f"rec{ch % 2}")
                            scr = tailp.tile([64, QB], f32, tag="scr", name=f"scr{ch % 2}")
                            nc.vector.reciprocal_approx_accurate(
                                rec[:, 0:cw], bc[:, 0:cw], scr[:, 0:cw]
                            )
                            dst = outT[
                                64 * hp : 64 * hp + 64,
                                hc,
                                c0 + ch * cw : c0 + (ch + 1) * cw,
                            ]
                            if hp == 0:
                                nc.vector.tensor_mul(dst, asb[0:HD, cl], rec[:, 0:cw])
                            else:
                                tmp = tailp.tile([64, QB], bf16, tag="tmp", name=f"tmp{ch % 2}")
                                nc.vector.tensor_mul(tmp[:, 0:cw], asb[0:HD, cl], rec[:, 0:cw])
                                nc.sync.dma_start(dst, tmp[:, 0:cw])

                    at_ring = {}
                    acc_cur = [None]
                    aptr = [0]

                    def do_attnv():
                        j = aptr[0]
                        half2, h2, kt2 = slots[j]
                        while kt2 >= v_kt_done[0]:
                            assert pump(), f"gen1 dry but V kt{kt2} missing"
                        if kt2 == 0:
                            acc_cur[0] = accp.tile([P, HB], f32, tag="acc", name="acc")
                        emit_attnv(acc_cur[0], h2, kt2, at_ring.pop(j))
                        if kt2 == NKT - 1:
                            emit_tail(
                                acc_cur[0], h2 % 2, h2 // 2, half2 * HB,
                                final=(j == NSLOT - 1),
                            )
                        aptr[0] += 1

                    for i in range(NSLOT):
                        # at-slot reuse safety before exp(i) overwrites it
                        while aptr[0] <= i - NRING:
                            do_attnv()
                        at_t = attnp.tile([P, HB], bf16, tag="attn", name=f"at{i % NRING}")
                        nc.scalar.activation(at_t[:], sct_ring.pop(i)[:], Exp)
                        at_ring[i] = at_t
                        if i + 2 < NSLOT:
                            sct_ring[i + 2] = scores_for_slot(i + 2)
                        budget = 2 if i < 40 else 1
                        for _ in range(budget):
                            if not pump():
                                break
                        if i >= 144:
                            for _ in range(2):
                                s = next(gen2, None)
                                if s is not None:
                                    s()
                        n = 0
                        while (
                            aptr[0] <= i - 2
                            and n < 3
                            and slots[aptr[0]][2] < v_kt_done[0]
                        ):
                            do_attnv()
                            n += 1

                    while aptr[0] < NSLOT:
                        do_attnv()
                    while pump():
                        pass
                    for s in gen2:
                        s()
                    drain_steps = oproj_steps

                # drain must not wait on the final softmax tail (HAM cold)
                fillp.release()
                scp_box[0].release()

                with (
                    tc.tile_pool(name="finp", bufs=4, space="PSUM") as finp,
                    tc.tile_pool(name="stg2", bufs=6) as stg2,
                ):
                    for s in drain_steps(range(8, 16), finp, stg2):
                        s()
                accp.release()

    nc.compile()
    return nc


def _get_prog():
    if _PROG[0] is None:
        _PROG[0] = _build()
    return _PROG[0]


def make_in_maps(query, key, value, Wq, Wk, Wv, Wo):
    import ml_dtypes

    bf16 = ml_dtypes.bfloat16
    scale = np.float32(1.0 / np.sqrt(D))
    Wq_s = (np.asarray(Wq, np.float32) * scale).astype(bf16)
    Wk_s = np.asarray(Wk, np.float32).astype(bf16)
    Wv_s = np.asarray(Wv, np.float32).astype(bf16)
    Wo_s = np.asarray(Wo, np.float32).astype(bf16)
    in_maps = []
    for b in range(B):
        xqT = np.ascontiguousarray(np.asarray(query[b], np.float32).T.astype(bf16))
        xkT = np.ascontiguousarray(np.asarray(key[b], np.float32).T.astype(bf16))
        xvT = np.ascontiguousarray(np.asarray(value[b], np.float32).T.astype(bf16))
        for hh in range(2):
            sl = slice(hh * DH, (hh + 1) * DH)
            in_maps.append(
                {
                    "xq": xqT,
                    "xk": xkT,
                    "xv": xvT,
                    "wq": np.ascontiguousarray(Wq_s[:, sl]),
                    "wk": np.ascontiguousarray(Wk_s[:, sl]),
                    "wv": np.ascontiguousarray(Wv_s[:, sl]),
                    "wo": np.ascontiguousarray(Wo_s[sl, :]),
                }
            )
    return in_maps


def run(in_maps, trace=False, **kw):
    from concourse.bass_utils import run_bass_kernel_spmd

    nc = _get_prog()
    return run_bass_kernel_spmd(
        nc, in_maps, core_ids=list(range(NCORES)), trace=trace, **kw
    )


def kernel(query, key, value, Wq, Wk, Wv, Wo, bo):
    in_maps = make_in_maps(query, key, value, Wq, Wk, Wv, Wo)
    res = run(in_maps)
    bo = np.asarray(bo, np.float32)
    out = np.empty((B, S, D), np.float32)
    for b in range(B):
        out[b] = (
            np.asarray(res.results[2 * b]["part"], np.float32)
            + np.asarray(res.results[2 * b + 1]["part"], np.float32)
            + bo
        )
    return out


# revision 37
# speedup vs baseline: 1.0321x; 1.0321x over previous
"""Multi-head attention (B=4, S=2048, D=1024, H=16) on 8 trn2 NeuronCores.

Sharding: batch (4-way) x head-half (2-way).  Core c = 2*b + hh handles batch b
and heads hh*8 .. hh*8+7.  All matmul operands are bf16; rel err ~6e-3.

  1. Prefix (~42us): load xk, project KT (q-block-paired, DMA-paced), load xq
     (reusing xk row slots), project QT q-half 0.  The V projection moves into
     the attention stream as filler steps; xv halves load during the prefix so
     V can start at stream slot 0.
  2. Scores run as a CONCURRENT row-tiled pair: KT2/QT2 shadow copies (DMA'd
     after each projection evacuation, partition halves swapped) let the j=1
     half read its operands from the opposite 64-partition strip, so both
     [64x128]x[64x512] matmuls occupy disjoint row groups of the PE array and
     execute simultaneously (~216ns/pair instead of ~530ns).  The freed PE
     slack absorbs the V projection + QT half-1 + half-0 output projection as
     in-stream fillers.
  3. attn@V may lag its exp slot (at-ring buffers the lag, emission forced
     before the ring wraps) until the V projection produces the needed token
     tile.  Softmax denominators ride as a ones-column in V (attnv M=65);
     per-head tails normalize via broadcast+reciprocal on the DVE.
  4. The half-1 output projection drains into PSUM freed by releasing the
     scores ring + filler pool but NOT the attnv accumulator pool, so the
     drain's pool boundary does not wait on the final softmax tail (that wait
     idled the PE >3.4us, HAM-throttled it, and ran the drain at half clock).
     Output partials are written bf16; the host sums them in fp32 + bias.
"""

import sys

if "/opt/trn_rl_repo" not in sys.path:
    sys.path.insert(0, "/opt/trn_rl_repo")

import numpy as np

B, S, D = 4, 2048, 1024
H, HD = 16, 64
P = 128
DK = D // P            # 8 contraction chunks for the projections
NKT = S // P           # 16 token tiles
QB = 512
DH = 512               # head dims per core (8 heads)
NDC = DH // P          # 4 dout chunks per core
NHC = 8                # heads per core
VW = HD + 1            # V columns per head incl. the ones column
HB = 1024              # q-half width
NCORES = 8
NRING = 16             # at-ring depth (attnv may lag exp by NRING-2 slots)

_PROG = [None]


def _build():
    import itertools

    import concourse.mybir as mybir
    import concourse.tile as tile
    from concourse import bacc

    f32 = mybir.dt.float32
    bf16 = mybir.dt.bfloat16
    Exp = mybir.ActivationFunctionType.Exp

    nc = bacc.Bacc("TRN2", target_bir_lowering=False, debug=False)
    xq = nc.dram_tensor("xq", [D, S], bf16, kind="ExternalInput").ap()
    xk = nc.dram_tensor("xk", [D, S], bf16, kind="ExternalInput").ap()
    xv = nc.dram_tensor("xv", [D, S], bf16, kind="ExternalInput").ap()
    wq = nc.dram_tensor("wq", [D, DH], bf16, kind="ExternalInput").ap()
    wk = nc.dram_tensor("wk", [D, DH], bf16, kind="ExternalInput").ap()
    wv = nc.dram_tensor("wv", [D, DH], bf16, kind="ExternalInput").ap()
    wo = nc.dram_tensor("wo", [DH, D], bf16, kind="ExternalInput").ap()
    part = nc.dram_tensor("part", [S, D], bf16, kind="ExternalOutput").ap()

    xq_v = xq.rearrange("(c p) s -> p c s", p=P)
    xk_v = xk.rearrange("(c p) s -> p c s", p=P)
    xv_v = xv.rearrange("(c p) s -> p c s", p=P)

    with tile.TileContext(nc) as tc:
        with tc.tile_pool(name="big", bufs=1) as big, tc.tile_pool(name="wp", bufs=4) as wp:
            QT = big.tile([P, NDC, S], bf16, tag="QT")
            KT = big.tile([P, NDC, S], bf16, tag="KT")
            QT2 = big.tile([P, NDC, S], bf16, tag="QT2")
            KT2 = big.tile([P, NDC, S], bf16, tag="KT2")
            V = big.tile([P, NKT, NHC * VW], bf16, tag="V")
            outT = big.tile([P, NDC, S], bf16, tag="outT")

            wk_t = wp.tile([P, DK, DH], bf16, tag="w", name="wk")
            wq_t = wp.tile([P, DK, DH], bf16, tag="w", name="wq")
            wv_t = wp.tile([P, DK, DH], bf16, tag="w", name="wv")
            wo_t = wp.tile([P, NDC, D], bf16, tag="w", name="wo")

            scp_box = [None]

            # warm the ACT exp table at t~0 (2.7us table load hides under DMA)
            with tc.tile_pool(name="wrm", bufs=1) as wrm:
                wc = wrm.tile([P, 8], f32, tag="wc")
                wout = wrm.tile([P, 8], bf16, tag="wo8")
                nc.vector.memset(wc[:], 0.0)
                nc.scalar.activation(wout[:], wc[:], Exp)

            def dma2(dst, src):
                nc.sync.dma_start(dst, src)

            def shadow(dst2, src, dc, c0, cw):
                # partition-half-swapped shadow copy (cross-partition: DMA)
                nc.sync.dma_start(dst2[0:64, dc, c0 : c0 + cw], src[64:128, dc, c0 : c0 + cw])
                nc.sync.dma_start(dst2[64:128, dc, c0 : c0 + cw], src[0:64, dc, c0 : c0 + cw])

            def emit_scores(kt, hc, r0, c0, use_shadow=True):
                # j=0 from the home strip, j=1 from the swapped shadow: the
                # two contraction-64 matmuls occupy disjoint PE row groups
                # and run concurrently.  The first slots skip the shadow so
                # the stream start never waits on the shadow DMAs.
                scp = scp_box[0]
                r1 = 64 - r0
                sct = scp.tile([P, HB], f32, tag="sc")
                nc.tensor.matmul(
                    sct[:, 0:QB],
                    KT[r0 : r0 + 64, hc, kt * P : (kt + 1) * P],
                    QT[r0 : r0 + 64, hc, c0 : c0 + QB],
                    start=True,
                    stop=True,
                )
                k2, q2, r2 = (KT2, QT2, r1) if use_shadow else (KT, QT, r0)
                nc.tensor.matmul(
                    sct[:, QB:HB],
                    k2[r2 : r2 + 64, hc, kt * P : (kt + 1) * P],
                    q2[r2 : r2 + 64, hc, c0 + QB : c0 + HB],
                    start=True,
                    stop=True,
                )
                return sct

            # hp=1 heads first: the final head's tail then has no DMA hop
            order = [1, 3, 5, 7, 0, 2, 4, 6]
            slots = [
                (half, h, kt)
                for half in (0, 1)
                for h in order
                for kt in range(NKT)
            ]
            NSLOT = len(slots)

            def scores_for_slot(i):
                half, h, kt = slots[i]
                return emit_scores(kt, h // 2, 64 * (h % 2), half * HB)

            # ---- prefix: KT (full) + QT half-0 ---------------------------
            with tc.tile_pool(name="xvp", bufs=8) as xvp:
                xr = tc.alloc_tile_pool(name="xr", bufs=8)
                xqr = tc.alloc_tile_pool(name="xqr", bufs=8)

                def load_w(w_t, w_dram):
                    w_v = w_dram.rearrange("(c p) m -> p c m", p=P)
                    for dk in range(DK):
                        nc.sync.dma_start(w_t[:, dk], w_v[:, dk])

                # all input DMAs emitted upfront, in priority order: the
                # queues then stream them back-to-back while the PE projects
                wk_v = wk.rearrange("(c p) m -> p c m", p=P)
                xk_rows = []
                for dk in range(DK):
                    nc.sync.dma_start(wk_t[:, dk], wk_v[:, dk])
                    xt = xr.tile([P, S], bf16, tag="xr", name=f"xr_k{dk}")
                    dma2(xt[:], xk_v[:, dk, :])
                    xk_rows.append(xt)
                wq_v = wq.rearrange("(c p) m -> p c m", p=P)
                xq_rows = []
                for dk in range(DK):
                    nc.sync.dma_start(wq_t[:, dk], wq_v[:, dk])
                    xt = xqr.tile([P, HB], bf16, tag="xq", name=f"xq{dk}")
                    dma2(xt[:], xq_v[:, dk, 0:HB])
                    xq_rows.append(xt)
                load_w(wv_t, wv)
                xvh = {0: [], 1: []}
                for h in (0, 1):
                    for dk in range(DK):
                        t = xvp.tile([P, HB], bf16, tag="xv", name=f"xv{h}_{dk}")
                        dma2(t[:], xv_v[:, dk, h * HB : (h + 1) * HB])
                        xvh[h].append(t)
                nc.sync.dma_start(wo_t[:], wo.rearrange("(c p) m -> p c m", p=P))

                # KT projection, q-block-paired to keep pace with the row DMA
                with tc.tile_pool(name="pp8", bufs=8, space="PSUM") as pp8:
                    for qbp in (0, 1):
                        pts = [
                            pp8.tile([P, QB], f32, tag="pp8", name=f"pk{i}")
                            for i in range(8)
                        ]
                        for dk in range(DK):
                            for u in range(2):
                                for dc in range(NDC):
                                    nc.tensor.matmul(
                                        pts[u * NDC + dc][:],
                                        wk_t[:, dk, dc * P : (dc + 1) * P],
                                        xk_rows[dk][
                                            :,
                                            (2 * qbp + u) * QB : (2 * qbp + u + 1) * QB,
                                        ],
                                        start=(dk == 0),
                                        stop=(dk == DK - 1),
                                    )
                        for u in range(2):
                            for dc in range(NDC):
                                c0 = (2 * qbp + u) * QB
                                dst = KT[:, dc, c0 : c0 + QB]
                                if dc % 2 == 0:
                                    nc.vector.tensor_copy(dst, pts[u * NDC + dc][:])
                                else:
                                    nc.scalar.copy(dst, pts[u * NDC + dc][:])
                                shadow(KT2, KT, dc, c0, QB)

                scp_box[0] = tc.alloc_tile_pool(
                    name="sc", bufs=2, space="PSUM", side="right"
                )
                pp = tc.alloc_tile_pool(name="pp", bufs=4, space="PSUM")
                # QT half-0 (q-blocks 0-1)
                for qb in (0, 1):
                    pts = [pp.tile([P, QB], f32, tag="pp", name=f"pp{i}") for i in range(NDC)]
                    for dk in range(DK):
                        for dc in range(NDC):
                            nc.tensor.matmul(
                                pts[dc][:],
                                wq_t[:, dk, dc * P : (dc + 1) * P],
                                xq_rows[dk][:, qb * QB : (qb + 1) * QB],
                                start=(dk == 0),
                                stop=(dk == DK - 1),
                            )
                    for dc in range(NDC):
                        c0 = qb * QB
                        dst = QT[:, dc, c0 : c0 + QB]
                        if dc % 2 == 0:
                            nc.vector.tensor_copy(dst, pts[dc][:])
                        else:
                            nc.scalar.copy(dst, pts[dc][:])
                        shadow(QT2, QT, dc, c0, QB)
                pp.release()
                xqr.release()
                xr.release()
                nc.vector.memset(V[:], 1.0)

                sct_ring = {0: scores_for_slot(0), 1: scores_for_slot(1)}

                # ---- attention stream with in-stream V projection --------
                accp = tc.alloc_tile_pool(name="acc", bufs=1, space="PSUM")
                fillp = tc.alloc_tile_pool(name="fillpp", bufs=2, space="PSUM")
                with (
                    tc.tile_pool(name="attn", bufs=NRING) as attnp,
                    tc.tile_pool(name="tail", bufs=1) as tailp,
                    tc.tile_pool(name="asbp", bufs=1) as asbp,
                    tc.tile_pool(name="stage", bufs=2) as stage,
                    tc.tile_pool(name="xf", bufs=8) as xf,
                ):
                    v_kt_done = [0]

                    def v_steps(qb):
                        """V projection q-block qb -> token tiles qb*4..+3."""
                        cur = {}
                        c0 = (qb % 2) * QB
                        for kt_in in range(4):
                            kt = qb * 4 + kt_in
                            def mm(kt_in, lo):
                                if lo == 0:
                                    cur[kt_in] = fillp.tile([P, DH], f32, tag="fp", name=f"fv{qb}_{kt_in}")
                                xs = xvh[qb // 2]
                                for dk in range(lo, lo + 4):
                                    nc.tensor.matmul(
                                        cur[kt_in][:],
                                        xs[dk][:, c0 + kt_in * P : c0 + (kt_in + 1) * P],
                                        wv_t[:, dk, :],
                                        start=(dk == 0),
                                        stop=(dk == DK - 1),
                                    )
                            def fin(kt_in=kt_in, kt=kt):
                                mm(kt_in, 4)
                                nc.vector.tensor_copy(
                                    V[:, kt].rearrange("p (h c) -> p h c", c=VW)[:, :, 0:HD],
                                    cur.pop(kt_in)[:].rearrange("p (h c) -> p h c", c=HD),
                                )
                                v_kt_done[0] = kt + 1
                            yield lambda kt_in=kt_in, mm=mm: mm(kt_in, 0)
                            yield fin

                    def qt23_steps():
                        """Project QT for q-half 1 (qb 2,3) + shadows."""
                        xts_all = {}
                        cur = {}

                        def dma_qb(dk0):
                            def go():
                                for dk in (dk0, dk0 + 1, dk0 + 2, dk0 + 3):
                                    xt = xf.tile([P, HB], bf16, tag="xf", name=f"xf{dk}")
                                    nc.sync.dma_start(
                                        xt[:], xq_v[:, dk, HB : 2 * HB]
                                    )
                                    xts_all[dk] = xt
                            return go

                        def mm_step(qb, dc, dk):
                            def go():
                                if dk == 0:
                                    cur[(qb, dc)] = fillp.tile([P, QB], f32, tag="fp", name=f"fq{qb}_{dc}")
                                nc.tensor.matmul(
                                    cur[(qb, dc)][:],
                                    wq_t[:, dk, dc * P : (dc + 1) * P],
                                    xts_all[dk][:, (qb - 2) * QB : (qb - 1) * QB],
                                    start=(dk == 0),
                                    stop=(dk == DK - 1),
                                )
                            return go

                        def copy_step(qb, dc):
                            def go():
                                c0 = qb * QB
                                nc.vector.tensor_copy(
                                    QT[:, dc, c0 : c0 + QB],
                                    cur[(qb, dc)][:],
                                )
                                shadow(QT2, QT, dc, c0, QB)
                            return go

                        yield dma_qb(0)
                        yield dma_qb(4)
                        for qb in (2, 3):
                            for dc in range(NDC):
                                for dk in range(DK):
                                    yield mm_step(qb, dc, dk)
                                yield copy_step(qb, dc)

                    def oproj_steps(qts, poolp=None, poolst=None):
                        pool_mm = poolp if poolp is not None else fillp
                        pool_st = poolst if poolst is not None else stage
                        cur = {}

                        def mm_step(qt, do, dc):
                            def go():
                                if dc == 0:
                                    cur[(qt, do)] = pool_mm.tile([P, QB], f32, tag="fp", name=f"fo{qt}_{do}")
                                nc.tensor.matmul(
                                    cur[(qt, do)][:],
                                    outT[:, dc, qt * P : (qt + 1) * P],
                                    wo_t[:, dc, do * QB : (do + 1) * QB],
                                    start=(dc == 0),
                                    stop=(dc == NDC - 1),
                                )
                            return go

                        def out_step(qt, do):
                            def go():
                                st = pool_st.tile([P, QB], bf16, tag="st", name=f"st{qt}_{do}")
                                nc.vector.tensor_copy(st[:], cur.pop((qt, do))[:])
                                nc.sync.dma_start(
                                    part[qt * P : (qt + 1) * P, do * QB : (do + 1) * QB],
                                    st[:],
                                )
                            return go

                        for qt in qts:
                            for do in range(2):
                                for dc in range(NDC):
                                    yield mm_step(qt, do, dc)
                                yield out_step(qt, do)

                    gen1 = itertools.chain(
                        v_steps(0), v_steps(1), v_steps(2), v_steps(3),
                        qt23_steps(),
                    )
                    gen2 = oproj_steps(range(8))

                    def pump():
                        s = next(gen1, None)
                        if s is None:
                            return False
                        s()
                        return True

                    def emit_attnv(acc, h, kt, at_t):
                        for j in range(2):
                            nc.tensor.matmul(
                                acc[0:VW, j * QB : (j + 1) * QB],
                                V[:, kt, h * VW : (h + 1) * VW],
                                at_t[:, j * QB : (j + 1) * QB],
                                start=(kt == 0),
                                stop=(kt == NKT - 1),
                            )

                    def emit_tail(acc, hp, hc, c0, final=False):
                        nch = 4 if final else 2
                        cw = HB // nch
                        asb = asbp.tile([96, HB], f32, tag="asb")
                        if not final:
                            nc.vector.tensor_copy(asb[0:VW, :], acc[0:VW, :])
                        for ch in range(nch):
                            cl = slice(ch * cw, (ch + 1) * cw)
                            if final:
                                nc.vector.tensor_copy(asb[0:VW, cl], acc[0:VW, cl])
                            bc = tailp.tile([64, QB], f32, tag="bc", name=f"bc{ch % 2}")
                            nc.vector.stream_shuffle(
                                bc[0:32, 0:cw], asb[64:96, cl], [0] * 32
                            )
                            nc.vector.stream_shuffle(
                                bc[32:64, 0:cw], asb[64:96, cl], [0] * 32
                            )
                            rec = tailp.tile([64, QB], f32, tag="rec", name=f"rec{ch % 2}")
                            scr = tailp.tile([64, QB], f32, tag="scr", name=f"scr{ch % 2}")
                            nc.vector.reciprocal_approx_accurate(
                                rec[:, 0:cw], bc[:, 0:cw], scr[:, 0:cw]
                            )
                            dst = outT[
                                64 * hp : 64 * hp + 64,
                                hc,
                                c0 + ch * cw : c0 + (ch + 1) * cw,
                            ]
                            if hp == 0:
                                nc.vector.tensor_mul(dst, asb[0:HD, cl], rec[:, 0:cw])
                            else:
                                tmp = tailp.tile([64, QB], bf16, tag="tmp", name=f"tmp{ch % 2}")
                                nc.vector.tensor_mul(tmp[:, 0:cw], asb[0:HD, cl], rec[:, 0:cw])
                                nc.sync.dma_start(dst, tmp[:, 0:cw])

                    at_ring = {}
                    acc_cur = [None]
                    aptr = [0]

                    def do_attnv():
                        j = aptr[0]
                        half2, h2, kt2 = slots[j]
                        while kt2 >= v_kt_done[0]:
                            assert pump(), f"gen1 dry but V kt{kt2} missing"
                        if kt2 == 0:
                            acc_cur[0] = accp.tile([P, HB], f32, tag="acc", name="acc")
                        emit_attnv(acc_cur[0], h2, kt2, at_ring.pop(j))
                        if kt2 == NKT - 1:
                            emit_tail(
                                acc_cur[0], h2 % 2, h2 // 2, half2 * HB,
                                final=(j == NSLOT - 1),
                            )
                        aptr[0] += 1

                    for i in range(NSLOT):
                        # at-slot reuse safety before exp(i) overwrites it
                        while aptr[0] <= i - NRING:
                            do_attnv()
                        at_t = attnp.tile([P, HB], bf16, tag="attn", name=f"at{i % NRING}")
                        nc.scalar.activation(at_t[:], sct_ring.pop(i)[:], Exp)
                        at_ring[i] = at_t
                        if i + 2 < NSLOT:
                            sct_ring[i + 2] = scores_for_slot(i + 2)
                        budget = 2 if i < 40 else 1
                        for _ in range(budget):
                            if not pump():
                                break
                        if i >= 144:
                            for _ in range(2):
                                s = next(gen2, None)
                                if s is not None:
                                    s()
                        n = 0
                        while (
                            aptr[0] <= i - 2
                            and n < 3
                            and slots[aptr[0]][2] < v_kt_done[0]
                        ):
                            do_attnv()
                            n += 1

                    while aptr[0] < NSLOT:
                        do_attnv()
                    while pump():
                        pass
                    for s in gen2:
                        s()
                    drain_steps = oproj_steps

                # drain must not wait on the final softmax tail (HAM cold)
                fillp.release()
                scp_box[0].release()

                with (
                    tc.tile_pool(name="finp", bufs=4, space="PSUM") as finp,
                    tc.tile_pool(name="stg2", bufs=6) as stg2,
                ):
                    for s in drain_steps(range(8, 16), finp, stg2):
                        s()
                accp.release()

    nc.compile()
    return nc


def _get_prog():
    if _PROG[0] is None:
        _PROG[0] = _build()
    return _PROG[0]


def make_in_maps(query, key, value, Wq, Wk, Wv, Wo):
    import ml_dtypes

    bf16 = ml_dtypes.bfloat16
    scale = np.float32(1.0 / np.sqrt(D))
    Wq_s = (np.asarray(Wq, np.float32) * scale).astype(bf16)
    Wk_s = np.asarray(Wk, np.float32).astype(bf16)
    Wv_s = np.asarray(Wv, np.float32).astype(bf16)
    Wo_s = np.asarray(Wo, np.float32).astype(bf16)
    in_maps = []
    for b in range(B):
        xqT = np.ascontiguousarray(np.asarray(query[b], np.float32).T.astype(bf16))
        xkT = np.ascontiguousarray(np.asarray(key[b], np.float32).T.astype(bf16))
        xvT = np.ascontiguousarray(np.asarray(value[b], np.float32).T.astype(bf16))
        for hh in range(2):
            sl = slice(hh * DH, (hh + 1) * DH)
            in_maps.append(
                {
                    "xq": xqT,
                    "xk": xkT,
                    "xv": xvT,
                    "wq": np.ascontiguousarray(Wq_s[:, sl]),
                    "wk": np.ascontiguousarray(Wk_s[:, sl]),
                    "wv": np.ascontiguousarray(Wv_s[:, sl]),
                    "wo": np.ascontiguousarray(Wo_s[sl, :]),
                }
            )
    return in_maps


def run(in_maps, trace=False, **kw):
    from concourse.bass_utils import run_bass_kernel_spmd

    nc = _get_prog()
    return run_bass_kernel_spmd(
        nc, in_maps, core_ids=list(range(NCORES)), trace=trace, **kw
    )


def kernel(query, key, value, Wq, Wk, Wv, Wo, bo):
    in_maps = make_in_maps(query, key, value, Wq, Wk, Wv, Wo)
    res = run(in_maps)
    bo = np.asarray(bo, np.float32)
    out = np.empty((B, S, D), np.float32)
    for b in range(B):
        out[b] = (
            np.asarray(res.results[2 * b]["part"], np.float32)
            + np.asarray(res.results[2 * b + 1]["part"], np.float32)
            + bo
        )
    return out


# revision 39
# speedup vs baseline: 1.0701x; 1.0368x over previous
"""Multi-head attention (B=4, S=2048, D=1024, H=16) on 8 trn2 NeuronCores.

Sharding: batch (4-way) x head-half (2-way).  Core c = 2*b + hh handles batch b
and heads hh*8 .. hh*8+7.  All matmul operands are bf16; rel err ~6e-3.

  1. Prefix (~42us): load xk, project KT (q-block-paired, DMA-paced), load xq
     (reusing xk row slots), project QT q-half 0.  The V projection moves into
     the attention stream as filler steps; xv halves load during the prefix so
     V can start at stream slot 0.
  2. Scores run as a CONCURRENT row-tiled pair: KT2/QT2 shadow copies (DMA'd
     after each projection evacuation, partition halves swapped) let the j=1
     half read its operands from the opposite 64-partition strip, so both
     [64x128]x[64x512] matmuls occupy disjoint row groups of the PE array and
     execute simultaneously (~216ns/pair instead of ~530ns).  The freed PE
     slack absorbs the V projection + QT half-1 + half-0 output projection as
     in-stream fillers.
  3. attn@V may lag its exp slot (at-ring buffers the lag, emission forced
     before the ring wraps) until the V projection produces the needed token
     tile.  Softmax denominators ride as a ones-column in V (attnv M=65);
     per-head tails normalize via broadcast+reciprocal on the DVE.
  4. The half-1 output projection drains into PSUM freed by releasing the
     scores ring + filler pool but NOT the attnv accumulator pool, so the
     drain's pool boundary does not wait on the final softmax tail (that wait
     idled the PE >3.4us, HAM-throttled it, and ran the drain at half clock).
     Output partials are written bf16; the host sums them in fp32 + bias.
"""

import sys

if "/opt/trn_rl_repo" not in sys.path:
    sys.path.insert(0, "/opt/trn_rl_repo")

import numpy as np

B, S, D = 4, 2048, 1024
H, HD = 16, 64
P = 128
DK = D // P            # 8 contraction chunks for the projections
NKT = S // P           # 16 token tiles
QB = 512
DH = 512               # head dims per core (8 heads)
NDC = DH // P          # 4 dout chunks per core
NHC = 8                # heads per core
VW = HD + 1            # V columns per head incl. the ones column
HB = 1024              # q-half width
NCORES = 8
NRING = 16             # at-ring depth (attnv may lag exp by NRING-2 slots)

_PROG = [None]


def _build():
    import itertools

    import concourse.mybir as mybir
    import concourse.tile as tile
    from concourse import bacc

    f32 = mybir.dt.float32
    bf16 = mybir.dt.bfloat16
    Exp = mybir.ActivationFunctionType.Exp

    nc = bacc.Bacc("TRN2", target_bir_lowering=False, debug=False)
    f8 = mybir.dt.float8e4
    xq = nc.dram_tensor("xq", [D, S], bf16, kind="ExternalInput").ap()
    xk = nc.dram_tensor("xk", [D // 2, 2 * S], f8, kind="ExternalInput").ap()
    xv = nc.dram_tensor("xv", [D, S], bf16, kind="ExternalInput").ap()
    wq = nc.dram_tensor("wq", [D, DH], bf16, kind="ExternalInput").ap()
    wk = nc.dram_tensor("wk", [D // 2, 2 * DH], f8, kind="ExternalInput").ap()
    wv = nc.dram_tensor("wv", [D, DH], bf16, kind="ExternalInput").ap()
    wo = nc.dram_tensor("wo", [DH, D], bf16, kind="ExternalInput").ap()
    part = nc.dram_tensor("part", [S, D], bf16, kind="ExternalOutput").ap()

    xq_v = xq.rearrange("(c p) s -> p c s", p=P)
    xk_v = xk.rearrange("(c p) (o s) -> p c o s", p=P, o=2)  # [P, 4, 2, S]
    xv_v = xv.rearrange("(c p) s -> p c s", p=P)

    with tile.TileContext(nc) as tc:
        with tc.tile_pool(name="big", bufs=1) as big, tc.tile_pool(name="wp", bufs=4) as wp:
            QT = big.tile([P, NDC, S], bf16, tag="QT")
            KT = big.tile([P, NDC, S], bf16, tag="KT")
            QT2 = big.tile([P, NDC, S], bf16, tag="QT2")
            KT2 = big.tile([P, NDC, S], bf16, tag="KT2")
            V = big.tile([P, NKT, NHC * VW], bf16, tag="V")
            outT = big.tile([P, NDC, S], bf16, tag="outT")

            wk_t = wp.tile([P, DK // 2, 2, DH], f8, tag="w", name="wk")
            wq_t = wp.tile([P, DK, DH], bf16, tag="w", name="wq")
            wv_t = wp.tile([P, DK, DH], bf16, tag="w", name="wv")
            wo_t = wp.tile([P, NDC, D], bf16, tag="w", name="wo")

            scp_box = [None]

            # warm the ACT exp table at t~0 (2.7us table load hides under DMA)
            with tc.tile_pool(name="wrm", bufs=1) as wrm:
                wc = wrm.tile([P, 8], f32, tag="wc")
                wout = wrm.tile([P, 8], bf16, tag="wo8")
                nc.vector.memset(wc[:], 0.0)
                nc.scalar.activation(wout[:], wc[:], Exp)

            def dma2(dst, src):
                nc.sync.dma_start(dst, src)

            def shadow(dst2, src, dc, c0, cw):
                # partition-half-swapped shadow copy (cross-partition: DMA)
                nc.sync.dma_start(dst2[0:64, dc, c0 : c0 + cw], src[64:128, dc, c0 : c0 + cw])
                nc.sync.dma_start(dst2[64:128, dc, c0 : c0 + cw], src[0:64, dc, c0 : c0 + cw])

            def emit_scores(kt, hc, r0, c0, use_shadow=True):
                # j=0 from the home strip, j=1 from the swapped shadow: the
                # two contraction-64 matmuls occupy disjoint PE row groups
                # and run concurrently.  The first slots skip the shadow so
                # the stream start never waits on the shadow DMAs.
                scp = scp_box[0]
                r1 = 64 - r0
                sct = scp.tile([P, HB], f32, tag="sc")
                nc.tensor.matmul(
                    sct[:, 0:QB],
                    KT[r0 : r0 + 64, hc, kt * P : (kt + 1) * P],
                    QT[r0 : r0 + 64, hc, c0 : c0 + QB],
                    start=True,
                    stop=True,
                )
                k2, q2, r2 = (KT2, QT2, r1) if use_shadow else (KT, QT, r0)
                nc.tensor.matmul(
                    sct[:, QB:HB],
                    k2[r2 : r2 + 64, hc, kt * P : (kt + 1) * P],
                    q2[r2 : r2 + 64, hc, c0 + QB : c0 + HB],
                    start=True,
                    stop=True,
                )
                return sct

            # hp=1 heads first: the final head's tail then has no DMA hop
            order = [1, 3, 5, 7, 0, 2, 4, 6]
            slots = [
                (half, h, kt)
                for half in (0, 1)
                for h in order
                for kt in range(NKT)
            ]
            NSLOT = len(slots)

            def scores_for_slot(i):
                half, h, kt = slots[i]
                return emit_scores(kt, h // 2, 64 * (h % 2), half * HB)

            # ---- prefix: KT (full) + QT half-0 ---------------------------
            with tc.tile_pool(name="xvp", bufs=8) as xvp:
                xr = tc.alloc_tile_pool(name="xr", bufs=8)
                xqr = tc.alloc_tile_pool(name="xqr", bufs=8)

                def load_w(w_t, w_dram):
                    w_v = w_dram.rearrange("(c p) m -> p c m", p=P)
                    for dk in range(DK):
                        nc.sync.dma_start(w_t[:, dk], w_v[:, dk])

                # all input DMAs emitted upfront, in priority order: the
                # queues then stream them back-to-back while the PE projects
                wk_v = wk.rearrange("(c p) (o m) -> p c o m", p=P, o=2)
                xk_rows = []
                for dk in range(DK // 2):
                    nc.sync.dma_start(wk_t[:, dk], wk_v[:, dk])
                    xt = xr.tile([P, 2, S], f8, tag="xr", name=f"xr_k{dk}")
                    dma2(xt[:], xk_v[:, dk])
                    xk_rows.append(xt)
                wq_v = wq.rearrange("(c p) m -> p c m", p=P)
                xq_rows = []
                for dk in range(DK):
                    nc.sync.dma_start(wq_t[:, dk], wq_v[:, dk])
                    xt = xqr.tile([P, HB], bf16, tag="xq", name=f"xq{dk}")
                    dma2(xt[:], xq_v[:, dk, 0:HB])
                    xq_rows.append(xt)
                load_w(wv_t, wv)
                xvh = {0: [], 1: []}
                for h in (0, 1):
                    for dk in range(DK):
                        t = xvp.tile([P, HB], bf16, tag="xv", name=f"xv{h}_{dk}")
                        dma2(t[:], xv_v[:, dk, h * HB : (h + 1) * HB])
                        xvh[h].append(t)
                nc.sync.dma_start(wo_t[:], wo.rearrange("(c p) m -> p c m", p=P))

                # KT projection, q-block-paired to keep pace with the row DMA
                with tc.tile_pool(name="pp8", bufs=8, space="PSUM") as pp8:
                    for qbp in (0, 1):
                        pts = [
                            pp8.tile([P, QB], f32, tag="pp8", name=f"pk{i}")
                            for i in range(8)
                        ]
                        for dk in range(DK // 2):
                            for u in range(2):
                                for dc in range(NDC):
                                    nc.tensor.matmul(
                                        pts[u * NDC + dc][:],
                                        wk_t[:, dk, :, dc * P : (dc + 1) * P],
                                        xk_rows[dk][
                                            :, :,
                                            (2 * qbp + u) * QB : (2 * qbp + u + 1) * QB,
                                        ],
                                        start=(dk == 0),
                                        stop=(dk == DK // 2 - 1),
                                        perf_mode=mybir.MatmulPerfMode.DoubleRow,
                                    )
                        for u in range(2):
                            for dc in range(NDC):
                                c0 = (2 * qbp + u) * QB
                                dst = KT[:, dc, c0 : c0 + QB]
                                if dc % 2 == 0:
                                    nc.vector.tensor_copy(dst, pts[u * NDC + dc][:])
                                else:
                                    nc.scalar.copy(dst, pts[u * NDC + dc][:])
                                shadow(KT2, KT, dc, c0, QB)

                scp_box[0] = tc.alloc_tile_pool(
                    name="sc", bufs=2, space="PSUM", side="right"
                )
                pp = tc.alloc_tile_pool(name="pp", bufs=4, space="PSUM")
                # QT half-0 (q-blocks 0-1)
                for qb in (0, 1):
                    pts = [pp.tile([P, QB], f32, tag="pp", name=f"pp{i}") for i in range(NDC)]
                    for dk in range(DK):
                        for dc in range(NDC):
                            nc.tensor.matmul(
                                pts[dc][:],
                                wq_t[:, dk, dc * P : (dc + 1) * P],
                                xq_rows[dk][:, qb * QB : (qb + 1) * QB],
                                start=(dk == 0),
                                stop=(dk == DK - 1),
                            )
                    for dc in range(NDC):
                        c0 = qb * QB
                        dst = QT[:, dc, c0 : c0 + QB]
                        if dc % 2 == 0:
                            nc.vector.tensor_copy(dst, pts[dc][:])
                        else:
                            nc.scalar.copy(dst, pts[dc][:])
                        shadow(QT2, QT, dc, c0, QB)
                pp.release()
                xqr.release()
                xr.release()
                nc.vector.memset(V[:], 1.0)

                sct_ring = {0: scores_for_slot(0), 1: scores_for_slot(1)}

                # ---- attention stream with in-stream V projection --------
                accp = tc.alloc_tile_pool(name="acc", bufs=1, space="PSUM")
                fillp = tc.alloc_tile_pool(name="fillpp", bufs=2, space="PSUM")
                with (
                    tc.tile_pool(name="attn", bufs=NRING) as attnp,
                    tc.tile_pool(name="tail", bufs=1) as tailp,
                    tc.tile_pool(name="asbp", bufs=1) as asbp,
                    tc.tile_pool(name="stage", bufs=2) as stage,
                    tc.tile_pool(name="xf", bufs=8) as xf,
                ):
                    v_kt_done = [0]

                    def v_steps(qb):
                        """V projection q-block qb -> token tiles qb*4..+3."""
                        cur = {}
                        c0 = (qb % 2) * QB
                        for kt_in in range(4):
                            kt = qb * 4 + kt_in
                            def mm(kt_in, lo):
                                if lo == 0:
                                    cur[kt_in] = fillp.tile([P, DH], f32, tag="fp", name=f"fv{qb}_{kt_in}")
                                xs = xvh[qb // 2]
                                for dk in range(lo, lo + 4):
                                    nc.tensor.matmul(
                                        cur[kt_in][:],
                                        xs[dk][:, c0 + kt_in * P : c0 + (kt_in + 1) * P],
                                        wv_t[:, dk, :],
                                        start=(dk == 0),
                                        stop=(dk == DK - 1),
                                    )
                            def fin(kt_in=kt_in, kt=kt):
                                mm(kt_in, 4)
                                nc.vector.tensor_copy(
                                    V[:, kt].rearrange("p (h c) -> p h c", c=VW)[:, :, 0:HD],
                                    cur.pop(kt_in)[:].rearrange("p (h c) -> p h c", c=HD),
                                )
                                v_kt_done[0] = kt + 1
                            yield lambda kt_in=kt_in, mm=mm: mm(kt_in, 0)
                            yield fin

                    def qt23_steps():
                        """Project QT for q-half 1 (qb 2,3) + shadows."""
                        xts_all = {}
                        cur = {}

                        def dma_qb(dk0):
                            def go():
                                for dk in (dk0, dk0 + 1, dk0 + 2, dk0 + 3):
                                    xt = xf.tile([P, HB], bf16, tag="xf", name=f"xf{dk}")
                                    nc.sync.dma_start(
                                        xt[:], xq_v[:, dk, HB : 2 * HB]
                                    )
                                    xts_all[dk] = xt
                            return go

                        def mm_step(qb, dc, dk):
                            def go():
                                if dk == 0:
                                    cur[(qb, dc)] = fillp.tile([P, QB], f32, tag="fp", name=f"fq{qb}_{dc}")
                                nc.tensor.matmul(
                                    cur[(qb, dc)][:],
                                    wq_t[:, dk, dc * P : (dc + 1) * P],
                                    xts_all[dk][:, (qb - 2) * QB : (qb - 1) * QB],
                                    start=(dk == 0),
                                    stop=(dk == DK - 1),
                                )
                            return go

                        def copy_step(qb, dc):
                            def go():
                                c0 = qb * QB
                                nc.vector.tensor_copy(
                                    QT[:, dc, c0 : c0 + QB],
                                    cur[(qb, dc)][:],
                                )
                                shadow(QT2, QT, dc, c0, QB)
                            return go

                        yield dma_qb(0)
                        yield dma_qb(4)
                        for qb in (2, 3):
                            for dc in range(NDC):
                                for dk in range(DK):
                                    yield mm_step(qb, dc, dk)
                                yield copy_step(qb, dc)

                    def oproj_steps(qts, poolp=None, poolst=None):
                        pool_mm = poolp if poolp is not None else fillp
                        pool_st = poolst if poolst is not None else stage
                        cur = {}

                        def mm_step(qt, do, dc):
                            def go():
                                if dc == 0:
                                    cur[(qt, do)] = pool_mm.tile([P, QB], f32, tag="fp", name=f"fo{qt}_{do}")
                                nc.tensor.matmul(
                                    cur[(qt, do)][:],
                                    outT[:, dc, qt * P : (qt + 1) * P],
                                    wo_t[:, dc, do * QB : (do + 1) * QB],
                                    start=(dc == 0),
                                    stop=(dc == NDC - 1),
                                )
                            return go

                        def out_step(qt, do):
                            def go():
                                st = pool_st.tile([P, QB], bf16, tag="st", name=f"st{qt}_{do}")
                                nc.vector.tensor_copy(st[:], cur.pop((qt, do))[:])
                                nc.sync.dma_start(
                                    part[qt * P : (qt + 1) * P, do * QB : (do + 1) * QB],
                                    st[:],
                                )
                            return go

                        for qt in qts:
                            for do in range(2):
                                for dc in range(NDC):
                                    yield mm_step(qt, do, dc)
                                yield out_step(qt, do)

                    gen1 = itertools.chain(
                        v_steps(0), v_steps(1), v_steps(2), v_steps(3),
                        qt23_steps(),
                    )
                    gen2 = oproj_steps(range(8))

                    def pump():
                        s = next(gen1, None)
                        if s is None:
                            return False
                        s()
                        return True

                    def emit_attnv(acc, h, kt, at_t):
                        for j in range(2):
                            nc.tensor.matmul(
                                acc[0:VW, j * QB : (j + 1) * QB],
                                V[:, kt, h * VW : (h + 1) * VW],
                                at_t[:, j * QB : (j + 1) * QB],
                                start=(kt == 0),
                                stop=(kt == NKT - 1),
                            )

                    def emit_tail(acc, hp, hc, c0, final=False):
                        nch = 4 if final else 2
                        cw = HB // nch
                        asb = asbp.tile([96, HB], f32, tag="asb")
                        if not final:
                            nc.vector.tensor_copy(asb[0:VW, :], acc[0:VW, :])
                        for ch in range(nch):
                            cl = slice(ch * cw, (ch + 1) * cw)
                            if final:
                                nc.vector.tensor_copy(asb[0:VW, cl], acc[0:VW, cl])
                            bc = tailp.tile([64, QB], f32, tag="bc", name=f"bc{ch % 2}")
                            nc.vector.stream_shuffle(
                                bc[0:32, 0:cw], asb[64:96, cl], [0] * 32
                            )
                            nc.vector.stream_shuffle(
                                bc[32:64, 0:cw], asb[64:96, cl], [0] * 32
                            )
                            rec = tailp.tile([64, QB], f32, tag="rec", name=f"rec{ch % 2}")
                            scr = tailp.tile([64, QB], f32, tag="scr", name=f"scr{ch % 2}")
                            nc.vector.reciprocal_approx_accurate(
                                rec[:, 0:cw], bc[:, 0:cw], scr[:, 0:cw]
                            )
                            dst = outT[
                                64 * hp : 64 * hp + 64,
                                hc,
                                c0 + ch * cw : c0 + (ch + 1) * cw,
                            ]
                            if hp == 0:
                                nc.vector.tensor_mul(dst, asb[0:HD, cl], rec[:, 0:cw])
                            else:
                                tmp = tailp.tile([64, QB], bf16, tag="tmp", name=f"tmp{ch % 2}")
                                nc.vector.tensor_mul(tmp[:, 0:cw], asb[0:HD, cl], rec[:, 0:cw])
                                nc.sync.dma_start(dst, tmp[:, 0:cw])

                    at_ring = {}
                    acc_cur = [None]
                    aptr = [0]

                    def do_attnv():
                        j = aptr[0]
                        half2, h2, kt2 = slots[j]
                        while kt2 >= v_kt_done[0]:
                            assert pump(), f"gen1 dry but V kt{kt2} missing"
                        if kt2 == 0:
                            acc_cur[0] = accp.tile([P, HB], f32, tag="acc", name="acc")
                        emit_attnv(acc_cur[0], h2, kt2, at_ring.pop(j))
                        if kt2 == NKT - 1:
                            emit_tail(
                                acc_cur[0], h2 % 2, h2 // 2, half2 * HB,
                                final=(j == NSLOT - 1),
                            )
                        aptr[0] += 1

                    for i in range(NSLOT):
                        # at-slot reuse safety before exp(i) overwrites it
                        while aptr[0] <= i - NRING:
                            do_attnv()
                        at_t = attnp.tile([P, HB], bf16, tag="attn", name=f"at{i % NRING}")
                        nc.scalar.activation(at_t[:], sct_ring.pop(i)[:], Exp)
                        at_ring[i] = at_t
                        if i + 2 < NSLOT:
                            sct_ring[i + 2] = scores_for_slot(i + 2)
                        budget = 2 if i < 40 else 1
                        for _ in range(budget):
                            if not pump():
                                break
                        if i >= 144:
                            for _ in range(2):
                                s = next(gen2, None)
                                if s is not None:
                                    s()
                        n = 0
                        while (
                            aptr[0] <= i - 2
                            and n < 3
                            and slots[aptr[0]][2] < v_kt_done[0]
                        ):
                            do_attnv()
                            n += 1

                    while aptr[0] < NSLOT:
                        do_attnv()
                    while pump():
                        pass
                    for s in gen2:
                        s()
                    drain_steps = oproj_steps

                # drain must not wait on the final softmax tail (HAM cold)
                fillp.release()
                scp_box[0].release()

                with (
                    tc.tile_pool(name="finp", bufs=4, space="PSUM") as finp,
                    tc.tile_pool(name="stg2", bufs=6) as stg2,
                ):
                    for s in drain_steps(range(8, 16), finp, stg2):
                        s()
                accp.release()

    nc.compile()
    return nc


def _get_prog():
    if _PROG[0] is None:
        _PROG[0] = _build()
    return _PROG[0]


def make_in_maps(query, key, value, Wq, Wk, Wv, Wo):
    import ml_dtypes

    bf16 = ml_dtypes.bfloat16
    f8 = ml_dtypes.float8_e4m3fn
    scale = np.float32(1.0 / np.sqrt(D))
    Wq_s = (np.asarray(Wq, np.float32) * scale / 32.0).astype(bf16)
    Wk_s = np.asarray(Wk, np.float32) * 32.0
    Wv_s = np.asarray(Wv, np.float32).astype(bf16)
    Wo_s = np.asarray(Wo, np.float32).astype(bf16)
    in_maps = []
    for b in range(B):
        xqT = np.ascontiguousarray(np.asarray(query[b], np.float32).T.astype(bf16))
        xkT = np.asarray(key[b], np.float32).T
        xk_dr = np.ascontiguousarray(
            xkT.reshape(4, 128, 2, S).reshape(D // 2, 2 * S)
        ).astype(f8)
        xvT = np.ascontiguousarray(np.asarray(value[b], np.float32).T.astype(bf16))
        for hh in range(2):
            sl = slice(hh * DH, (hh + 1) * DH)
            in_maps.append(
                {
                    "xq": xqT,
                    "xk": xk_dr,
                    "xv": xvT,
                    "wq": np.ascontiguousarray(Wq_s[:, sl]),
                    "wk": np.ascontiguousarray(
                        Wk_s[:, sl].reshape(4, 128, 2, DH).reshape(D // 2, 2 * DH)
                    ).astype(f8),
                    "wv": np.ascontiguousarray(Wv_s[:, sl]),
                    "wo": np.ascontiguousarray(Wo_s[sl, :]),
                }
            )
    return in_maps


def run(in_maps, trace=False, **kw):
    from concourse.bass_utils import run_bass_kernel_spmd

    nc = _get_prog()
    return run_bass_kernel_spmd(
        nc, in_maps, core_ids=list(range(NCORES)), trace=trace, **kw
    )


def kernel(query, key, value, Wq, Wk, Wv, Wo, bo):
    in_maps = make_in_maps(query, key, value, Wq, Wk, Wv, Wo)
    res = run(in_maps)
    bo = np.asarray(bo, np.float32)
    out = np.empty((B, S, D), np.float32)
    for b in range(B):
        out[b] = (
            np.asarray(res.results[2 * b]["part"], np.float32)
            + np.asarray(res.results[2 * b + 1]["part"], np.float32)
            + bo
        )
    return out


# revision 40
# speedup vs baseline: 1.0758x; 1.0053x over previous
"""Multi-head attention (B=4, S=2048, D=1024, H=16) on 8 trn2 NeuronCores.

Sharding: batch (4-way) x head-half (2-way).  Core c = 2*b + hh handles batch b
and heads hh*8 .. hh*8+7.  All matmul operands are bf16; rel err ~6e-3.

  1. Prefix (~42us): load xk, project KT (q-block-paired, DMA-paced), load xq
     (reusing xk row slots), project QT q-half 0.  The V projection moves into
     the attention stream as filler steps; xv halves load during the prefix so
     V can start at stream slot 0.
  2. Scores run as a CONCURRENT row-tiled pair: KT2/QT2 shadow copies (DMA'd
     after each projection evacuation, partition halves swapped) let the j=1
     half read its operands from the opposite 64-partition strip, so both
     [64x128]x[64x512] matmuls occupy disjoint row groups of the PE array and
     execute simultaneously (~216ns/pair instead of ~530ns).  The freed PE
     slack absorbs the V projection + QT half-1 + half-0 output projection as
     in-stream fillers.
  3. attn@V may lag its exp slot (at-ring buffers the lag, emission forced
     before the ring wraps) until the V projection produces the needed token
     tile.  Softmax denominators ride as a ones-column in V (attnv M=65);
     per-head tails normalize via broadcast+reciprocal on the DVE.
  4. The half-1 output projection drains into PSUM freed by releasing the
     scores ring + filler pool but NOT the attnv accumulator pool, so the
     drain's pool boundary does not wait on the final softmax tail (that wait
     idled the PE >3.4us, HAM-throttled it, and ran the drain at half clock).
     Output partials are written bf16; the host sums them in fp32 + bias.
"""

import sys

if "/opt/trn_rl_repo" not in sys.path:
    sys.path.insert(0, "/opt/trn_rl_repo")

import numpy as np

B, S, D = 4, 2048, 1024
H, HD = 16, 64
P = 128
DK = D // P            # 8 contraction chunks for the projections
NKT = S // P           # 16 token tiles
QB = 512
DH = 512               # head dims per core (8 heads)
NDC = DH // P          # 4 dout chunks per core
NHC = 8                # heads per core
VW = HD + 1            # V columns per head incl. the ones column
HB = 1024              # q-half width
NCORES = 8
NRING = 16             # at-ring depth (attnv may lag exp by NRING-2 slots)

_PROG = [None]


def _build():
    import itertools

    import concourse.mybir as mybir
    import concourse.tile as tile
    from concourse import bacc

    f32 = mybir.dt.float32
    bf16 = mybir.dt.bfloat16
    Exp = mybir.ActivationFunctionType.Exp

    nc = bacc.Bacc("TRN2", target_bir_lowering=False, debug=False)
    f8 = mybir.dt.float8e4
    xq = nc.dram_tensor("xq", [D, S], bf16, kind="ExternalInput").ap()
    xk = nc.dram_tensor("xk", [D // 2, 2 * S], f8, kind="ExternalInput").ap()
    xv = nc.dram_tensor("xv", [D, S], bf16, kind="ExternalInput").ap()
    wq = nc.dram_tensor("wq", [D, DH], bf16, kind="ExternalInput").ap()
    wk = nc.dram_tensor("wk", [D // 2, 2 * DH], f8, kind="ExternalInput").ap()
    wv = nc.dram_tensor("wv", [D, DH], bf16, kind="ExternalInput").ap()
    wo = nc.dram_tensor("wo", [DH, D], bf16, kind="ExternalInput").ap()
    part = nc.dram_tensor("part", [S, D], bf16, kind="ExternalOutput").ap()

    xq_v = xq.rearrange("(c p) s -> p c s", p=P)
    xk_v = xk.rearrange("(c p) (o s) -> p c o s", p=P, o=2)  # [P, 4, 2, S]
    xv_v = xv.rearrange("(c p) s -> p c s", p=P)

    with tile.TileContext(nc) as tc:
        with tc.tile_pool(name="big", bufs=1) as big, tc.tile_pool(name="wp", bufs=4) as wp:
            QT = big.tile([P, NDC, S], bf16, tag="QT")
            KT = big.tile([P, NDC, S], bf16, tag="KT")
            QT2 = big.tile([P, NDC, S], bf16, tag="QT2")
            KT2 = big.tile([P, NDC, S], bf16, tag="KT2")
            V = big.tile([P, NKT, NHC * VW], bf16, tag="V")
            outT = big.tile([P, NDC, S], bf16, tag="outT")

            wk_t = wp.tile([P, DK // 2, 2, DH], f8, tag="w", name="wk")
            wq_t = wp.tile([P, DK, DH], bf16, tag="w", name="wq")
            wv_t = wp.tile([P, DK, DH], bf16, tag="w", name="wv")
            wo_t = wp.tile([P, NDC, D], bf16, tag="w", name="wo")

            scp_box = [None]

            # warm the ACT exp table at t~0 (2.7us table load hides under DMA)
            with tc.tile_pool(name="wrm", bufs=1) as wrm:
                wc = wrm.tile([P, 8], f32, tag="wc")
                wout = wrm.tile([P, 8], bf16, tag="wo8")
                nc.vector.memset(wc[:], 0.0)
                nc.scalar.activation(wout[:], wc[:], Exp)

            def dma2(dst, src):
                nc.sync.dma_start(dst, src)

            def shadow(dst2, src, dc, c0, cw):
                # partition-half-swapped shadow copy (cross-partition: DMA)
                nc.sync.dma_start(dst2[0:64, dc, c0 : c0 + cw], src[64:128, dc, c0 : c0 + cw])
                nc.sync.dma_start(dst2[64:128, dc, c0 : c0 + cw], src[0:64, dc, c0 : c0 + cw])

            def emit_scores(kt, hc, r0, c0, use_shadow=True):
                # j=0 from the home strip, j=1 from the swapped shadow: the
                # two contraction-64 matmuls occupy disjoint PE row groups
                # and run concurrently.  The first slots skip the shadow so
                # the stream start never waits on the shadow DMAs.
                scp = scp_box[0]
                r1 = 64 - r0
                sct = scp.tile([P, HB], f32, tag="sc")
                nc.tensor.matmul(
                    sct[:, 0:QB],
                    KT[r0 : r0 + 64, hc, kt * P : (kt + 1) * P],
                    QT[r0 : r0 + 64, hc, c0 : c0 + QB],
                    start=True,
                    stop=True,
                )
                k2, q2, r2 = (KT2, QT2, r1) if use_shadow else (KT, QT, r0)
                nc.tensor.matmul(
                    sct[:, QB:HB],
                    k2[r2 : r2 + 64, hc, kt * P : (kt + 1) * P],
                    q2[r2 : r2 + 64, hc, c0 + QB : c0 + HB],
                    start=True,
                    stop=True,
                )
                return sct

            # hp=1 heads first: the final head's tail then has no DMA hop
            order = [1, 3, 5, 7, 0, 2, 4, 6]
            slots = [
                (half, h, kt)
                for half in (0, 1)
                for h in order
                for kt in range(NKT)
            ]
            NSLOT = len(slots)

            def scores_for_slot(i):
                half, h, kt = slots[i]
                # shadows land ~20us after the projections (DMA-queued behind
                # the inputs); early slots run the j=1 half serial from the
                # originals so the exp stream starts immediately
                return emit_scores(
                    kt, h // 2, 64 * (h % 2), half * HB, use_shadow=(i >= 24)
                )

            # ---- prefix: KT (full) + QT half-0 ---------------------------
            with tc.tile_pool(name="xvp", bufs=8) as xvp:
                xr = tc.alloc_tile_pool(name="xr", bufs=8)
                xqr = tc.alloc_tile_pool(name="xqr", bufs=8)

                def load_w(w_t, w_dram):
                    w_v = w_dram.rearrange("(c p) m -> p c m", p=P)
                    for dk in range(DK):
                        nc.sync.dma_start(w_t[:, dk], w_v[:, dk])

                # all input DMAs emitted upfront, in priority order: the
                # queues then stream them back-to-back while the PE projects
                wk_v = wk.rearrange("(c p) (o m) -> p c o m", p=P, o=2)
                xk_rows = []
                for dk in range(DK // 2):
                    nc.sync.dma_start(wk_t[:, dk], wk_v[:, dk])
                    xt = xr.tile([P, 2, S], f8, tag="xr", name=f"xr_k{dk}")
                    dma2(xt[:], xk_v[:, dk])
                    xk_rows.append(xt)
                wq_v = wq.rearrange("(c p) m -> p c m", p=P)
                xq_rows = []
                for dk in range(DK):
                    nc.sync.dma_start(wq_t[:, dk], wq_v[:, dk])
                    xt = xqr.tile([P, HB], bf16, tag="xq", name=f"xq{dk}")
                    dma2(xt[:], xq_v[:, dk, 0:HB])
                    xq_rows.append(xt)
                load_w(wv_t, wv)
                xvh = {0: [], 1: []}
                for h in (0, 1):
                    for dk in range(DK):
                        t = xvp.tile([P, HB], bf16, tag="xv", name=f"xv{h}_{dk}")
                        dma2(t[:], xv_v[:, dk, h * HB : (h + 1) * HB])
                        xvh[h].append(t)

                # KT projection, q-block-paired to keep pace with the row DMA
                with tc.tile_pool(name="pp8", bufs=8, space="PSUM") as pp8:
                    for qbp in (0, 1):
                        pts = [
                            pp8.tile([P, QB], f32, tag="pp8", name=f"pk{i}")
                            for i in range(8)
                        ]
                        for dk in range(DK // 2):
                            for u in range(2):
                                for dc in range(NDC):
                                    nc.tensor.matmul(
                                        pts[u * NDC + dc][:],
                                        wk_t[:, dk, :, dc * P : (dc + 1) * P],
                                        xk_rows[dk][
                                            :, :,
                                            (2 * qbp + u) * QB : (2 * qbp + u + 1) * QB,
                                        ],
                                        start=(dk == 0),
                                        stop=(dk == DK // 2 - 1),
                                        perf_mode=mybir.MatmulPerfMode.DoubleRow,
                                    )
                        for u in range(2):
                            for dc in range(NDC):
                                c0 = (2 * qbp + u) * QB
                                dst = KT[:, dc, c0 : c0 + QB]
                                if dc % 2 == 0:
                                    nc.vector.tensor_copy(dst, pts[u * NDC + dc][:])
                                else:
                                    nc.scalar.copy(dst, pts[u * NDC + dc][:])
                                shadow(KT2, KT, dc, c0, QB)

                scp_box[0] = tc.alloc_tile_pool(
                    name="sc", bufs=2, space="PSUM", side="right"
                )
                pp = tc.alloc_tile_pool(name="pp", bufs=4, space="PSUM")
                # QT half-0 (q-blocks 0-1)
                for qb in (0, 1):
                    pts = [pp.tile([P, QB], f32, tag="pp", name=f"pp{i}") for i in range(NDC)]
                    for dk in range(DK):
                        for dc in range(NDC):
                            nc.tensor.matmul(
                                pts[dc][:],
                                wq_t[:, dk, dc * P : (dc + 1) * P],
                                xq_rows[dk][:, qb * QB : (qb + 1) * QB],
                                start=(dk == 0),
                                stop=(dk == DK - 1),
                            )
                    for dc in range(NDC):
                        c0 = qb * QB
                        dst = QT[:, dc, c0 : c0 + QB]
                        if dc % 2 == 0:
                            nc.vector.tensor_copy(dst, pts[dc][:])
                        else:
                            nc.scalar.copy(dst, pts[dc][:])
                        shadow(QT2, QT, dc, c0, QB)
                pp.release()
                xqr.release()
                xr.release()
                nc.sync.dma_start(wo_t[:], wo.rearrange("(c p) m -> p c m", p=P))
                nc.vector.memset(V[:], 1.0)

                sct_ring = {0: scores_for_slot(0), 1: scores_for_slot(1)}

                # ---- attention stream with in-stream V projection --------
                accp = tc.alloc_tile_pool(name="acc", bufs=1, space="PSUM")
                fillp = tc.alloc_tile_pool(name="fillpp", bufs=2, space="PSUM")
                with (
                    tc.tile_pool(name="attn", bufs=NRING) as attnp,
                    tc.tile_pool(name="tail", bufs=1) as tailp,
                    tc.tile_pool(name="asbp", bufs=1) as asbp,
                    tc.tile_pool(name="stage", bufs=2) as stage,
                    tc.tile_pool(name="xf", bufs=8) as xf,
                ):
                    v_kt_done = [0]

                    def v_steps(qb):
                        """V projection q-block qb -> token tiles qb*4..+3."""
                        cur = {}
                        c0 = (qb % 2) * QB
                        for kt_in in range(4):
                            kt = qb * 4 + kt_in
                            def mm(kt_in, lo):
                                if lo == 0:
                                    cur[kt_in] = fillp.tile([P, DH], f32, tag="fp", name=f"fv{qb}_{kt_in}")
                                xs = xvh[qb // 2]
                                for dk in range(lo, lo + 4):
                                    nc.tensor.matmul(
                                        cur[kt_in][:],
                                        xs[dk][:, c0 + kt_in * P : c0 + (kt_in + 1) * P],
                                        wv_t[:, dk, :],
                                        start=(dk == 0),
                                        stop=(dk == DK - 1),
                                    )
                            def fin(kt_in=kt_in, kt=kt):
                                mm(kt_in, 4)
                                nc.vector.tensor_copy(
                                    V[:, kt].rearrange("p (h c) -> p h c", c=VW)[:, :, 0:HD],
                                    cur.pop(kt_in)[:].rearrange("p (h c) -> p h c", c=HD),
                                )
                                v_kt_done[0] = kt + 1
                            yield lambda kt_in=kt_in, mm=mm: mm(kt_in, 0)
                            yield fin

                    def qt23_steps():
                        """Project QT for q-half 1 (qb 2,3) + shadows."""
                        xts_all = {}
                        cur = {}

                        def dma_qb(dk0):
                            def go():
                                for dk in (dk0, dk0 + 1, dk0 + 2, dk0 + 3):
                                    xt = xf.tile([P, HB], bf16, tag="xf", name=f"xf{dk}")
                                    nc.sync.dma_start(
                                        xt[:], xq_v[:, dk, HB : 2 * HB]
                                    )
                                    xts_all[dk] = xt
                            return go

                        def mm_step(qb, dc, dk):
                            def go():
                                if dk == 0:
                                    cur[(qb, dc)] = fillp.tile([P, QB], f32, tag="fp", name=f"fq{qb}_{dc}")
                                nc.tensor.matmul(
                                    cur[(qb, dc)][:],
                                    wq_t[:, dk, dc * P : (dc + 1) * P],
                                    xts_all[dk][:, (qb - 2) * QB : (qb - 1) * QB],
                                    start=(dk == 0),
                                    stop=(dk == DK - 1),
                                )
                            return go

                        def copy_step(qb, dc):
                            def go():
                                c0 = qb * QB
                                nc.vector.tensor_copy(
                                    QT[:, dc, c0 : c0 + QB],
                                    cur[(qb, dc)][:],
                                )
                                shadow(QT2, QT, dc, c0, QB)
                            return go

                        yield dma_qb(0)
                        yield dma_qb(4)
                        for qb in (2, 3):
                            for dc in range(NDC):
                                for dk in range(DK):
                                    yield mm_step(qb, dc, dk)
                                yield copy_step(qb, dc)

                    def oproj_steps(qts, poolp=None, poolst=None):
                        pool_mm = poolp if poolp is not None else fillp
                        pool_st = poolst if poolst is not None else stage
                        cur = {}

                        def mm_step(qt, do, dc):
                            def go():
                                if dc == 0:
                                    cur[(qt, do)] = pool_mm.tile([P, QB], f32, tag="fp", name=f"fo{qt}_{do}")
                                nc.tensor.matmul(
                                    cur[(qt, do)][:],
                                    outT[:, dc, qt * P : (qt + 1) * P],
                                    wo_t[:, dc, do * QB : (do + 1) * QB],
                                    start=(dc == 0),
                                    stop=(dc == NDC - 1),
                                )
                            return go

                        def out_step(qt, do):
                            def go():
                                st = pool_st.tile([P, QB], bf16, tag="st", name=f"st{qt}_{do}")
                                nc.vector.tensor_copy(st[:], cur.pop((qt, do))[:])
                                nc.sync.dma_start(
                                    part[qt * P : (qt + 1) * P, do * QB : (do + 1) * QB],
                                    st[:],
                                )
                            return go

                        for qt in qts:
                            for do in range(2):
                                for dc in range(NDC):
                                    yield mm_step(qt, do, dc)
                                yield out_step(qt, do)

                    gen1 = itertools.chain(
                        v_steps(0), v_steps(1), v_steps(2), v_steps(3),
                        qt23_steps(),
                    )
                    gen2 = oproj_steps(range(8))

                    def pump():
                        s = next(gen1, None)
                        if s is None:
                            return False
                        s()
                        return True

                    def emit_attnv(acc, h, kt, at_t):
                        for j in range(2):
                            nc.tensor.matmul(
                                acc[0:VW, j * QB : (j + 1) * QB],
                                V[:, kt, h * VW : (h + 1) * VW],
                                at_t[:, j * QB : (j + 1) * QB],
                                start=(kt == 0),
                                stop=(kt == NKT - 1),
                            )

                    def emit_tail(acc, hp, hc, c0, final=False):
                        nch = 4 if final else 2
                        cw = HB // nch
                        asb = asbp.tile([96, HB], f32, tag="asb")
                        if not final:
                            nc.vector.tensor_copy(asb[0:VW, :], acc[0:VW, :])
                        for ch in range(nch):
                            cl = slice(ch * cw, (ch + 1) * cw)
                            if final:
                                nc.vector.tensor_copy(asb[0:VW, cl], acc[0:VW, cl])
                            bc = tailp.tile([64, QB], f32, tag="bc", name=f"bc{ch % 2}")
                            nc.vector.stream_shuffle(
                                bc[0:32, 0:cw], asb[64:96, cl], [0] * 32
                            )
                            nc.vector.stream_shuffle(
                                bc[32:64, 0:cw], asb[64:96, cl], [0] * 32
                            )
                            rec = tailp.tile([64, QB], f32, tag="rec", name=f"rec{ch % 2}")
                            scr = tailp.tile([64, QB], f32, tag="scr", name=f"scr{ch % 2}")
                            nc.vector.reciprocal_approx_accurate(
                                rec[:, 0:cw], bc[:, 0:cw], scr[:, 0:cw]
                            )
                            dst = outT[
                                64 * hp : 64 * hp + 64,
                                hc,
                                c0 + ch * cw : c0 + (ch + 1) * cw,
                            ]
                            if hp == 0:
                                nc.vector.tensor_mul(dst, asb[0:HD, cl], rec[:, 0:cw])
                            else:
                                tmp = tailp.tile([64, QB], bf16, tag="tmp", name=f"tmp{ch % 2}")
                                nc.vector.tensor_mul(tmp[:, 0:cw], asb[0:HD, cl], rec[:, 0:cw])
                                nc.sync.dma_start(dst, tmp[:, 0:cw])

                    at_ring = {}
                    acc_cur = [None]
                    aptr = [0]

                    def do_attnv():
                        j = aptr[0]
                        half2, h2, kt2 = slots[j]
                        while kt2 >= v_kt_done[0]:
                            assert pump(), f"gen1 dry but V kt{kt2} missing"
                        if kt2 == 0:
                            acc_cur[0] = accp.tile([P, HB], f32, tag="acc", name="acc")
                        emit_attnv(acc_cur[0], h2, kt2, at_ring.pop(j))
                        if kt2 == NKT - 1:
                            emit_tail(
                                acc_cur[0], h2 % 2, h2 // 2, half2 * HB,
                                final=(j == NSLOT - 1),
                            )
                        aptr[0] += 1

                    for i in range(NSLOT):
                        # at-slot reuse safety before exp(i) overwrites it
                        while aptr[0] <= i - NRING:
                            do_attnv()
                        at_t = attnp.tile([P, HB], bf16, tag="attn", name=f"at{i % NRING}")
                        nc.scalar.activation(at_t[:], sct_ring.pop(i)[:], Exp)
                        at_ring[i] = at_t
                        if i + 2 < NSLOT:
                            sct_ring[i + 2] = scores_for_slot(i + 2)
                        budget = 2 if i < 40 else 1
                        for _ in range(budget):
                            if not pump():
                                break
                        if i >= 144:
                            for _ in range(2):
                                s = next(gen2, None)
                                if s is not None:
                                    s()
                        n = 0
                        while (
                            aptr[0] <= i - 2
                            and n < 3
                            and slots[aptr[0]][2] < v_kt_done[0]
                        ):
                            do_attnv()
                            n += 1

                    while aptr[0] < NSLOT:
                        do_attnv()
                    while pump():
                        pass
                    for s in gen2:
                        s()
                    drain_steps = oproj_steps

                # drain must not wait on the final softmax tail (HAM cold)
                fillp.release()
                scp_box[0].release()

                with (
                    tc.tile_pool(name="finp", bufs=4, space="PSUM") as finp,
                    tc.tile_pool(name="stg2", bufs=6) as stg2,
                ):
                    for s in drain_steps(range(8, 16), finp, stg2):
                        s()
                accp.release()

    nc.compile()
    return nc


def _get_prog():
    if _PROG[0] is None:
        _PROG[0] = _build()
    return _PROG[0]


def make_in_maps(query, key, value, Wq, Wk, Wv, Wo):
    import ml_dtypes

    bf16 = ml_dtypes.bfloat16
    f8 = ml_dtypes.float8_e4m3fn
    scale = np.float32(1.0 / np.sqrt(D))
    Wq_s = (np.asarray(Wq, np.float32) * scale / 32.0).astype(bf16)
    Wk_s = np.asarray(Wk, np.float32) * 32.0
    Wv_s = np.asarray(Wv, np.float32).astype(bf16)
    Wo_s = np.asarray(Wo, np.float32).astype(bf16)
    in_maps = []
    for b in range(B):
        xqT = np.ascontiguousarray(np.asarray(query[b], np.float32).T.astype(bf16))
        xkT = np.asarray(key[b], np.float32).T
        xk_dr = np.ascontiguousarray(
            xkT.reshape(4, 128, 2, S).reshape(D // 2, 2 * S)
        ).astype(f8)
        xvT = np.ascontiguousarray(np.asarray(value[b], np.float32).T.astype(bf16))
        for hh in range(2):
            sl = slice(hh * DH, (hh + 1) * DH)
            in_maps.append(
                {
                    "xq": xqT,
                    "xk": xk_dr,
                    "xv": xvT,
                    "wq": np.ascontiguousarray(Wq_s[:, sl]),
                    "wk": np.ascontiguousarray(
                        Wk_s[:, sl].reshape(4, 128, 2, DH).reshape(D // 2, 2 * DH)
                    ).astype(f8),
                    "wv": np.ascontiguousarray(Wv_s[:, sl]),
                    "wo": np.ascontiguousarray(Wo_s[sl, :]),
                }
            )
    return in_maps


def run(in_maps, trace=False, **kw):
    from concourse.bass_utils import run_bass_kernel_spmd

    nc = _get_prog()
    return run_bass_kernel_spmd(
        nc, in_maps, core_ids=list(range(NCORES)), trace=trace, **kw
    )


def kernel(query, key, value, Wq, Wk, Wv, Wo, bo):
    in_maps = make_in_maps(query, key, value, Wq, Wk, Wv, Wo)
    res = run(in_maps)
    bo = np.asarray(bo, np.float32)
    out = np.empty((B, S, D), np.float32)
    for b in range(B):
        out[b] = (
            np.asarray(res.results[2 * b]["part"], np.float32)
            + np.asarray(res.results[2 * b + 1]["part"], np.float32)
            + bo
        )
    return out
